# revision 12
# baseline (speedup 1.0000x reference)
"""GCN layer relu(GCNConv(x, edge_index)) on 8 Trainium2 NeuronCores.

Math (PyG GCNConv with self-loops, symmetric norm):
    deg[v]  = 1 + in-degree(v)
    s       = deg ** -0.5
    out[d]  = relu(s[d] * (sum_{e: dst(e)=d} s[src_e] * (x[src_e] @ W)) + b)

Two-launch transform-first pipeline (D_OUT=32 << D_IN=128, so messages
shrink 4x when the linear transform runs before the gather):

  Pass 1 (device): per core, its 12544 nodes' features arrive
    feature-major ([128, NPOS] fp16); 98 matmuls against W and a fused
    PSUM-drain multiply by s_own (deg**-0.5, computed on device) yield
    h' = s * (x @ W) in fp16, written back to HBM along with s_own.

  Host (index bookkeeping only): scatter h' rows to node order, gather
    them into each core's degree-sorted padded ELL slot table
    (slot 0 of each node = its self-loop).  No host float arithmetic -
    rows are moved verbatim; s[src] is already folded into h'.

  Pass 2 (device): stream the gathered messages ([128, totk*32] fp16,
    ~15 MB/core vs 56 MB/core when gathering x), segment-sum each
    node's K slots with a pairwise tensor_tensor ADD tree (2x DVE mode
    on contiguous fp16; tensor_reduce only has a 1x uop), then
    relu(s_own * sum + b) and stream out per subgroup.

Indirect DMA is deliberately avoided: TRN2's dynamic DMA honors only
one runtime offset per partition per instruction, far too slow for
1.7M edge gathers.  Replicating h' per edge costs a 4x smaller (and
perfectly sequential) HBM stream than replicating x.
"""

import math
import numpy as np

import concourse.bass as bass
import concourse.bacc as bacc
import concourse.mybir as mybir
import concourse.tile as tile
from concourse import bass_utils

# ---------------------------------------------------------------- config ---
P = 128            # partitions
D_IN = 128
D_OUT = 32
N = 100000         # nodes
E = 1600000        # edges
NCORES = 8

NPC = N // NCORES              # 12500 nodes per core
TPC = math.ceil(NPC / P)       # 98 node tiles per core
NPOS = TPC * P                 # 12544 padded positions per core
NPAD0 = NPOS - NPC             # 44 pad positions (front, degree 0)
NV = NCORES * NPOS             # padded global positions

XCH = 14                       # node tiles per pass-1 matmul/drain chunk (98 = 7*14)
DMA_COLS = 1024                # pass-2 he stream chunk (2 KiB/partition)
NTK_MAX = 256                  # max nt*K per pass-2 subgroup (16 KiB stage)

F16 = mybir.dt.float16
F32 = mybir.dt.float32


# ------------------------------------------------------------- host prep ---
def graph_prep(edge_index):
    """Degree-sorted node placement + padded ELL slot table (ints only)."""
    src = np.asarray(edge_index[0]).astype(np.int64)
    dst = np.asarray(edge_index[1]).astype(np.int64)
    deg = np.bincount(dst, minlength=N).astype(np.int64) + 1   # + self loop

    node_of_pos = np.full(NV, -1, dtype=np.int64)
    pos_of_node = np.empty(N, dtype=np.int64)
    for c in range(NCORES):
        lo = c * NPC
        order = np.argsort(deg[lo:lo + NPC], kind="stable")
        qs = c * NPOS + NPAD0 + np.arange(NPC)
        node_of_pos[qs] = lo + order
        pos_of_node[lo + order] = qs

    sdeg = np.zeros(NV, dtype=np.int64)
    valid = node_of_pos >= 0
    sdeg[valid] = deg[node_of_pos[valid]]

    # Per-tile slot count K_t (shared across cores; SPMD: one program),
    # rounded up to a multiple of 2 (cheap zero-padding, fewer K runs).
    ktile = sdeg.reshape(NCORES, TPC, P).max(axis=(0, 2))
    ktile = (np.maximum(ktile, 1) + 1) // 2 * 2
    offs = np.concatenate([[0], np.cumsum(ktile)]).astype(np.int64)
    totk = int(offs[-1])

    # slot source table: src_slot[core][p, c] = source node of that slot
    # (-1 for padding).  Slot offs[t]+0 of node (t,p) is its self loop.
    src_slot = np.full((NCORES, P, totk), -1, dtype=np.int64)
    vreal = np.nonzero(valid)[0]
    rp = vreal % P
    rt = (vreal % NPOS) // P
    rc = vreal // NPOS
    src_slot[rc, rp, offs[rt]] = node_of_pos[vreal]          # self slots

    key = pos_of_node[dst]
    es = np.argsort(key, kind="stable")
    key_s = key[es]
    src_s = src[es]
    newrun = np.ones(E, dtype=bool)
    newrun[1:] = key_s[1:] != key_s[:-1]
    run_start = np.maximum.accumulate(np.where(newrun, np.arange(E), 0))
    kwith = np.arange(E) - run_start + 1
    ep = key_s % P
    et = (key_s % NPOS) // P
    ec = key_s // NPOS
    src_slot[ec, ep, offs[et] + kwith] = src_s

    # own-node degree per (p, t) for s = deg**-0.5 (pads get 1 -> s=1)
    dego = np.ones((NCORES, P, TPC), dtype=np.float16)
    sd = sdeg.reshape(NCORES, TPC, P)
    for c in range(NCORES):
        dego[c] = np.maximum(sd[c].T, 1).astype(np.float16)

    return deg, node_of_pos, src_slot, dego, ktile, offs, totk


def subgroups(ktile):
    """Runs of equal K, split so nt*K <= NTK_MAX (stage tile bound)."""
    sg = []
    t0 = 0
    while t0 < TPC:
        t1 = t0 + 1
        while t1 < TPC and ktile[t1] == ktile[t0]:
            t1 += 1
        k = int(ktile[t0])
        ntmax = max(1, NTK_MAX // k)
        s = t0
        while s < t1:
            e = min(t1, s + ntmax)
            sg.append((s, e, k))
            s = e
        t0 = t1
    return sg


# ------------------------------------------------------------ pass 1 ---
def build_pass1():
    """h' = s_own * (x @ W) per core; also exports s_own."""
    nc = bacc.Bacc(None, num_devices=NCORES)

    xT = nc.dram_tensor("xT", [P, NPOS], F16, kind="ExternalInput")
    w = nc.dram_tensor("w", [P, D_OUT], F16, kind="ExternalInput")
    dego = nc.dram_tensor("dego", [P, TPC], F16, kind="ExternalInput")
    hp = nc.dram_tensor("hp", [P, TPC * D_OUT], F16, kind="ExternalOutput")
    sown = nc.dram_tensor("sown", [P, TPC], F32, kind="ExternalOutput")

    pieces = [0, 28, 56, 84, TPC]   # xT arrives in 4 large DMAs

    with tile.TileContext(nc) as tc:
        with (
            tc.tile_pool(name="const", bufs=1) as cpool,
            tc.tile_pool(name="psum", bufs=4, space="PSUM") as psum_pool,
        ):
            w_sb = cpool.tile([P, D_OUT], F16)
            dego_sb = cpool.tile([P, TPC], F16)
            rtmp = cpool.tile([P, TPC], F32)
            s_own = cpool.tile([P, TPC], F32)
            hsb = cpool.tile([P, TPC * D_OUT], F16)
            xsb = cpool.tile([P, NPOS], F16)

            nc.sync.dma_start(out=w_sb[:], in_=w[:, :])
            nc.sync.dma_start(out=dego_sb[:], in_=dego[:, :])
            for a, b in zip(pieces, pieces[1:]):
                nc.sync.dma_start(
                    out=xsb[:, a * P:b * P], in_=xT[:, a * P:b * P]
                )

            # s = deg ** -0.5 (recip on DVE, sqrt on ACT)
            nc.vector.reciprocal(out=rtmp[:], in_=dego_sb[:])
            nc.scalar.sqrt(out=s_own[:], in_=rtmp[:])
            nc.sync.dma_start(out=sown[:, :], in_=s_own[:])

            for c0 in range(0, TPC, XCH):
                ch = min(XCH, TPC - c0)
                ps = psum_pool.tile([P, XCH * D_OUT], F32, tag="ps")
                for j in range(ch):
                    nc.tensor.matmul(
                        out=ps[:, j * D_OUT:(j + 1) * D_OUT],
                        lhsT=xsb[:, (c0 + j) * P:(c0 + j + 1) * P],
                        rhs=w_sb[:],
                        start=True,
                        stop=True,
                    )
                # fused PSUM drain: h' = ps * s_own  (f16 out)
                nc.vector.tensor_tensor(
                    out=hsb[:, c0 * D_OUT:(c0 + ch) * D_OUT]
                    .rearrange("p (t f) -> p t f", f=D_OUT),
                    in0=ps[:, :ch * D_OUT].rearrange("p (t f) -> p t f", f=D_OUT),
                    in1=s_own[:, c0:c0 + ch].to_broadcast([P, ch, D_OUT]),
                    op=mybir.AluOpType.mult,
                )
                if c0 + ch in pieces:
                    a = pieces[pieces.index(c0 + ch) - 1]
                    nc.sync.dma_start(
                        out=hp[:, a * D_OUT:(c0 + ch) * D_OUT],
                        in_=hsb[:, a * D_OUT:(c0 + ch) * D_OUT],
                    )

    nc.finalize()
    return nc


# ------------------------------------------------------------ pass 2 ---
def build_pass2(ktile, offs, totk, with_bias):
    """Segment-sum gathered messages, then relu(s_own * sum + b).

    with_bias=False elides the bias adds (adding an all-zero b is an
    identity); the generic path stays available for nonzero b.
    """
    nc = bacc.Bacc(None, num_devices=NCORES)

    he = nc.dram_tensor("he", [P, totk * D_OUT], F16, kind="ExternalInput")
    sown = nc.dram_tensor("sown", [P, TPC], F32, kind="ExternalInput")
    bias = nc.dram_tensor("bias", [P, D_OUT], F32, kind="ExternalInput")
    out = nc.dram_tensor("out", [P, TPC * D_OUT], F32, kind="ExternalOutput")

    sg = subgroups(ktile)
    stg_cols = max((t1 - t0) * k for t0, t1, k in sg) * D_OUT
    scra_cols = max((t1 - t0) * (k // 2 + 1) for t0, t1, k in sg) * D_OUT
    scrb_cols = max((t1 - t0) * (k // 4 + 2) for t0, t1, k in sg) * D_OUT

    with tile.TileContext(nc) as tc:
        with (
            tc.tile_pool(name="const", bufs=1) as cpool,
            tc.tile_pool(name="stage", bufs=4) as spool,
            tc.tile_pool(name="scra", bufs=2) as apool,
            tc.tile_pool(name="scrb", bufs=2) as bpool,
        ):
            sown_sb = cpool.tile([P, TPC], F32)
            bias_sb = cpool.tile([P, D_OUT], F32)
            tbuf = cpool.tile([P, TPC * D_OUT], F32)

            nc.sync.dma_start(out=sown_sb[:], in_=sown[:, :])
            nc.sync.dma_start(out=bias_sb[:], in_=bias[:, :])

            for (t0, t1, k) in sg:
                nt = t1 - t0
                base = int(offs[t0]) * D_OUT
                wcols = nt * k * D_OUT
                stg = spool.tile([P, stg_cols], F16, tag="stg")
                nc.sync.dma_start(
                    out=stg[:, :wcols], in_=he[:, base:base + wcols]
                )

                # pairwise halving tree over the k slots of each node
                cur, kc, toggle = stg, k, 0
                while kc > 1:
                    half = kc // 2
                    odd = kc & 1
                    newk = half + odd
                    cur_v = cur[:, :nt * kc * D_OUT].rearrange(
                        "p (t k f) -> p t k f", k=kc, f=D_OUT
                    )
                    if newk == 1:      # kc == 2: final add straight to f32
                        dst_v = tbuf[:, t0 * D_OUT:t1 * D_OUT].rearrange(
                            "p (t f) -> p t f", f=D_OUT
                        ).unsqueeze(2)
                    else:
                        pool = apool if toggle == 0 else bpool
                        cols = apool is pool and scra_cols or scrb_cols
                        dst = pool.tile(
                            [P, cols], F16, tag="sa" if toggle == 0 else "sb"
                        )
                        dst_v = dst[:, :nt * newk * D_OUT].rearrange(
                            "p (t k f) -> p t k f", k=newk, f=D_OUT
                        )
                    nc.vector.tensor_tensor(
                        out=dst_v[:, :, 0:half, :],
                        in0=cur_v[:, :, 0:half, :],
                        in1=cur_v[:, :, half:2 * half, :],
                        op=mybir.AluOpType.add,
                    )
                    if odd:
                        nc.scalar.copy(
                            out=dst_v[:, :, half:newk, :],
                            in_=cur_v[:, :, 2 * half:kc, :],
                        )
                    if newk > 1:
                        cur, toggle = dst, 1 - toggle
                    kc = newk

                # epilogue: out = relu(s_own * sum + b)
                t3 = tbuf[:, t0 * D_OUT:t1 * D_OUT].rearrange(
                    "p (t f) -> p t f", f=D_OUT
                )
                nc.vector.tensor_tensor(
                    out=t3, in0=t3,
                    in1=sown_sb[:, t0:t1].to_broadcast([P, nt, D_OUT]),
                    op=mybir.AluOpType.mult,
                )
                if with_bias:
                    bias_b = bass.AP(
                        bias_sb[:].tensor, bias_sb[:].offset,
                        [[D_OUT, P], [0, nt], [1, D_OUT]],
                    )
                    nc.vector.tensor_tensor(
                        out=t3, in0=t3, in1=bias_b, op=mybir.AluOpType.add
                    )
                nc.scalar.activation(
                    out=tbuf[:, t0 * D_OUT:t1 * D_OUT],
                    in_=tbuf[:, t0 * D_OUT:t1 * D_OUT],
                    func=mybir.ActivationFunctionType.Relu,
                )
                nc.sync.dma_start(
                    out=out[:, t0 * D_OUT:t1 * D_OUT],
                    in_=tbuf[:, t0 * D_OUT:t1 * D_OUT],
                )

    nc.finalize()
    return nc


# ---------------------------------------------------------------- runner ---
def _run(inputs, trace=False):
    x, edge_index = inputs["x"], inputs["edge_index"]
    W, b = inputs["W"], inputs["b"]
    deg, node_of_pos, src_slot, dego, ktile, offs, totk = graph_prep(edge_index)

    # ---- pass 1: h' = s * (x @ W) per core
    x16 = np.concatenate(
        [np.asarray(x).astype(np.float16), np.zeros((1, D_IN), np.float16)]
    )
    w16 = np.asarray(W).astype(np.float16)
    nop_safe = np.where(node_of_pos >= 0, node_of_pos, N)
    nc1 = build_pass1()
    in1 = [
        {
            "xT": np.ascontiguousarray(
                x16[nop_safe[c * NPOS:(c + 1) * NPOS]].T
            ),
            "w": w16,
            "dego": dego[c],
        }
        for c in range(NCORES)
    ]
    res1 = bass_utils.run_bass_kernel_spmd(
        nc1, in1, core_ids=list(range(NCORES)), trace=trace
    )

    # ---- host: gather h' rows into the ELL slot layout (no float math)
    h_aug = np.zeros((N + 1, D_OUT), dtype=np.float16)
    for c in range(NCORES):
        hp = res1.results[c]["hp"].reshape(P, TPC, D_OUT)
        block = hp.transpose(1, 0, 2).reshape(NPOS, D_OUT)
        nid = node_of_pos[c * NPOS:(c + 1) * NPOS]
        m = nid >= 0
        h_aug[nid[m]] = block[m]
    slot_safe = np.where(src_slot >= 0, src_slot, N)

    # ---- pass 2: segment-sum + epilogue
    bias = np.broadcast_to(np.asarray(b).astype(np.float32), (P, D_OUT)).copy()
    with_bias = bool(np.any(np.asarray(b) != 0))
    nc2 = build_pass2(ktile, offs, totk, with_bias)
    in2 = [
        {
            "he": h_aug[slot_safe[c]].reshape(P, totk * D_OUT),
            "sown": res1.results[c]["sown"],
            "bias": bias,
        }
        for c in range(NCORES)
    ]
    res2 = bass_utils.run_bass_kernel_spmd(
        nc2, in2, core_ids=list(range(NCORES)), trace=trace
    )

    # ---- unshard
    full = np.empty((N, D_OUT), dtype=np.float32)
    for c in range(NCORES):
        oc = res2.results[c]["out"].reshape(P, TPC, D_OUT)
        block = oc.transpose(1, 0, 2).reshape(NPOS, D_OUT)
        nid = node_of_pos[c * NPOS:(c + 1) * NPOS]
        m = nid >= 0
        full[nid[m]] = block[m]
    return full, [res1, res2]


def kernel(**inputs) -> np.ndarray:
    full, _ = _run(inputs, trace=False)
    return full


# revision 18
# speedup vs baseline: 1.0669x; 1.0669x over previous
"""GCN layer relu(GCNConv(x, edge_index)) on 8 Trainium2 NeuronCores.

Math (PyG GCNConv with self-loops, symmetric norm):
    deg[v]  = 1 + in-degree(v)
    s       = deg ** -0.5
    out[d]  = relu(s[d] * (sum_{e: dst(e)=d} s[src_e] * (x[src_e] @ W)) + b)

Two-launch transform-first pipeline (D_OUT=32 << D_IN=128, so messages
shrink 4x when the linear transform runs before the gather):

  Pass 1 (device): per core, its 12544 nodes' features arrive
    feature-major ([128, NPOS] fp16); 98 matmuls against W and a fused
    PSUM-drain multiply by s_own (deg**-0.5, computed on device) yield
    h' = s * (x @ W) in fp16, written back to HBM along with s_own.

  Host (index bookkeeping only): scatter h' rows to node order, gather
    them into each core's degree-sorted padded ELL slot table
    (slot 0 of each node = its self-loop).  No host float arithmetic -
    rows are moved verbatim; s[src] is already folded into h'.

  Pass 2 (device): stream the gathered messages ([128, totk*32] fp16,
    ~15 MB/core vs 56 MB/core when gathering x), segment-sum each
    node's K slots with a pairwise tensor_tensor ADD tree (2x DVE mode
    on contiguous fp16; tensor_reduce only has a 1x uop), then
    relu(s_own * sum + b) and stream out per subgroup.

Indirect DMA is deliberately avoided: TRN2's dynamic DMA honors only
one runtime offset per partition per instruction, far too slow for
1.7M edge gathers.  Replicating h' per edge costs a 4x smaller (and
perfectly sequential) HBM stream than replicating x.
"""

import math
import numpy as np

import concourse.bass as bass
import concourse.bacc as bacc
import concourse.mybir as mybir
import concourse.tile as tile
from concourse import bass_utils

# ---------------------------------------------------------------- config ---
P = 128            # partitions
D_IN = 128
D_OUT = 32
N = 100000         # nodes
E = 1600000        # edges
NCORES = 8

NPC = N // NCORES              # 12500 nodes per core
TPC = math.ceil(NPC / P)       # 98 node tiles per core
NPOS = TPC * P                 # 12544 padded positions per core
NPAD0 = NPOS - NPC             # 44 pad positions (front, degree 0)
NV = NCORES * NPOS             # padded global positions

XCH = 14                       # node tiles per pass-1 matmul/drain chunk (98 = 7*14)
DMA_COLS = 1024                # pass-2 he stream chunk (2 KiB/partition)
NTK_MAX = 256                  # max nt*K per pass-2 subgroup (16 KiB stage)

F16 = mybir.dt.float16
F32 = mybir.dt.float32


# ------------------------------------------------------------- host prep ---
def graph_prep(edge_index):
    """Degree-sorted node placement + padded ELL slot table (ints only)."""
    src = np.asarray(edge_index[0]).astype(np.int64)
    dst = np.asarray(edge_index[1]).astype(np.int64)
    deg = np.bincount(dst, minlength=N).astype(np.int64) + 1   # + self loop

    node_of_pos = np.full(NV, -1, dtype=np.int64)
    pos_of_node = np.empty(N, dtype=np.int64)
    for c in range(NCORES):
        lo = c * NPC
        order = np.argsort(deg[lo:lo + NPC], kind="stable")
        qs = c * NPOS + NPAD0 + np.arange(NPC)
        node_of_pos[qs] = lo + order
        pos_of_node[lo + order] = qs

    sdeg = np.zeros(NV, dtype=np.int64)
    valid = node_of_pos >= 0
    sdeg[valid] = deg[node_of_pos[valid]]

    # Per-tile slot count K_t (shared across cores; SPMD: one program),
    # rounded up to a multiple of 2 (cheap zero-padding, fewer K runs).
    # Runs shorter than 3 tiles are padded up into the next run: fewer,
    # larger DMAs and subgroups beat the few extra zero slots.
    ktile = sdeg.reshape(NCORES, TPC, P).max(axis=(0, 2))
    ktile = (np.maximum(ktile, 1) + 1) // 2 * 2
    while True:
        runs = []
        t0 = 0
        while t0 < TPC:
            t1 = t0 + 1
            while t1 < TPC and ktile[t1] == ktile[t0]:
                t1 += 1
            runs.append((t0, t1))
            t0 = t1
        short = [i for i, (a, b) in enumerate(runs[:-1]) if b - a < 3]
        if not short:
            break
        a, b = runs[short[0]]
        ktile[a:b] = ktile[b]          # pad up into the next (larger) run
    offs = np.concatenate([[0], np.cumsum(ktile)]).astype(np.int64)
    totk = int(offs[-1])

    # slot source table: src_slot[core][p, c] = source node of that slot
    # (-1 for padding).  Slot offs[t]+0 of node (t,p) is its self loop.
    src_slot = np.full((NCORES, P, totk), -1, dtype=np.int64)
    vreal = np.nonzero(valid)[0]
    rp = vreal % P
    rt = (vreal % NPOS) // P
    rc = vreal // NPOS
    src_slot[rc, rp, offs[rt]] = node_of_pos[vreal]          # self slots

    key = pos_of_node[dst]
    es = np.argsort(key, kind="stable")
    key_s = key[es]
    src_s = src[es]
    newrun = np.ones(E, dtype=bool)
    newrun[1:] = key_s[1:] != key_s[:-1]
    run_start = np.maximum.accumulate(np.where(newrun, np.arange(E), 0))
    kwith = np.arange(E) - run_start + 1
    ep = key_s % P
    et = (key_s % NPOS) // P
    ec = key_s // NPOS
    src_slot[ec, ep, offs[et] + kwith] = src_s

    # own-node degree per (p, t) for s = deg**-0.5 (pads get 1 -> s=1)
    dego = np.ones((NCORES, P, TPC), dtype=np.float16)
    sd = sdeg.reshape(NCORES, TPC, P)
    for c in range(NCORES):
        dego[c] = np.maximum(sd[c].T, 1).astype(np.float16)

    return deg, node_of_pos, src_slot, dego, ktile, offs, totk


def subgroups(ktile):
    """Runs of equal K, split so nt*K <= NTK_MAX (stage tile bound)."""
    sg = []
    t0 = 0
    while t0 < TPC:
        t1 = t0 + 1
        while t1 < TPC and ktile[t1] == ktile[t0]:
            t1 += 1
        k = int(ktile[t0])
        ntmax = max(1, NTK_MAX // k)
        s = t0
        while s < t1:
            e = min(t1, s + ntmax)
            sg.append((s, e, k))
            s = e
        t0 = t1
    return sg


# ------------------------------------------------------------ pass 1 ---
def build_pass1():
    """h' = s_own * (x @ W) per core; also exports s_own."""
    nc = bacc.Bacc(None, num_devices=NCORES)

    xT = nc.dram_tensor("xT", [P, NPOS], F16, kind="ExternalInput")
    w = nc.dram_tensor("w", [P, D_OUT], F16, kind="ExternalInput")
    dego = nc.dram_tensor("dego", [P, TPC], F16, kind="ExternalInput")
    hp = nc.dram_tensor("hp", [P, TPC * D_OUT], F16, kind="ExternalOutput")
    sown = nc.dram_tensor("sown", [P, TPC], F32, kind="ExternalOutput")

    pieces = [0, 28, 56, 84, TPC]   # xT arrives in 4 large DMAs

    with tile.TileContext(nc) as tc:
        with (
            tc.tile_pool(name="const", bufs=1) as cpool,
            tc.tile_pool(name="psum", bufs=7, space="PSUM") as psum_pool,
        ):
            w_sb = cpool.tile([P, D_OUT], F16)
            dego_sb = cpool.tile([P, TPC], F16)
            rtmp = cpool.tile([P, TPC], F32)
            s_own = cpool.tile([P, TPC], F32)
            hsb = cpool.tile([P, TPC * D_OUT], F16)
            xsb = cpool.tile([P, NPOS], F16)

            nc.sync.dma_start(out=w_sb[:], in_=w[:, :])
            nc.sync.dma_start(out=dego_sb[:], in_=dego[:, :])
            for a, b in zip(pieces, pieces[1:]):
                nc.sync.dma_start(
                    out=xsb[:, a * P:b * P], in_=xT[:, a * P:b * P]
                )

            # s = deg ** -0.5 (recip on DVE, sqrt on ACT)
            nc.vector.reciprocal(out=rtmp[:], in_=dego_sb[:])
            nc.scalar.sqrt(out=s_own[:], in_=rtmp[:])
            nc.sync.dma_start(out=sown[:, :], in_=s_own[:])

            for c0 in range(0, TPC, XCH):
                ch = min(XCH, TPC - c0)
                ps = psum_pool.tile([P, XCH * D_OUT], F32, tag="ps")
                for j in range(ch):
                    nc.tensor.matmul(
                        out=ps[:, j * D_OUT:(j + 1) * D_OUT],
                        lhsT=xsb[:, (c0 + j) * P:(c0 + j + 1) * P],
                        rhs=w_sb[:],
                        start=True,
                        stop=True,
                    )
                # fused PSUM drain: h' = ps * s_own  (f16 out)
                nc.vector.tensor_tensor(
                    out=hsb[:, c0 * D_OUT:(c0 + ch) * D_OUT]
                    .rearrange("p (t f) -> p t f", f=D_OUT),
                    in0=ps[:, :ch * D_OUT].rearrange("p (t f) -> p t f", f=D_OUT),
                    in1=s_own[:, c0:c0 + ch].to_broadcast([P, ch, D_OUT]),
                    op=mybir.AluOpType.mult,
                )
                if c0 + ch in pieces:
                    a = pieces[pieces.index(c0 + ch) - 1]
                    nc.sync.dma_start(
                        out=hp[:, a * D_OUT:(c0 + ch) * D_OUT],
                        in_=hsb[:, a * D_OUT:(c0 + ch) * D_OUT],
                    )

    nc.finalize()
    return nc


# ------------------------------------------------------------ pass 2 ---
def build_pass2(ktile, offs, totk, with_bias):
    """Segment-sum gathered messages, then relu(s_own * sum + b).

    with_bias=False elides the bias adds (adding an all-zero b is an
    identity); the generic path stays available for nonzero b.
    """
    nc = bacc.Bacc(None, num_devices=NCORES)

    he = nc.dram_tensor("he", [P, totk * D_OUT], F16, kind="ExternalInput")
    sown = nc.dram_tensor("sown", [P, TPC], F32, kind="ExternalInput")
    bias = nc.dram_tensor("bias", [P, D_OUT], F32, kind="ExternalInput")
    out = nc.dram_tensor("out", [P, TPC * D_OUT], F16, kind="ExternalOutput")

    sg = subgroups(ktile)
    stg_cols = max((t1 - t0) * k for t0, t1, k in sg) * D_OUT
    scra_cols = max((t1 - t0) * (k // 2 + 1) for t0, t1, k in sg) * D_OUT
    scrb_cols = max((t1 - t0) * (k // 4 + 2) for t0, t1, k in sg) * D_OUT

    with tile.TileContext(nc) as tc:
        with (
            tc.tile_pool(name="const", bufs=1) as cpool,
            tc.tile_pool(name="stage", bufs=5) as spool,
            tc.tile_pool(name="scra", bufs=2) as apool,
            tc.tile_pool(name="scrb", bufs=2) as bpool,
        ):
            sown_sb = cpool.tile([P, TPC], F32)
            bias_sb = cpool.tile([P, D_OUT], F32)
            tbuf = cpool.tile([P, TPC * D_OUT], F32)
            obuf = cpool.tile([P, TPC * D_OUT], F16)

            nc.sync.dma_start(out=sown_sb[:], in_=sown[:, :])
            nc.sync.dma_start(out=bias_sb[:], in_=bias[:, :])

            for (t0, t1, k) in sg:
                nt = t1 - t0
                base = int(offs[t0]) * D_OUT
                wcols = nt * k * D_OUT
                stg = spool.tile([P, stg_cols], F16, tag="stg")
                nc.sync.dma_start(
                    out=stg[:, :wcols], in_=he[:, base:base + wcols]
                )

                # pairwise halving tree over the k slots of each node
                cur, kc, toggle = stg, k, 0
                while kc > 1:
                    half = kc // 2
                    odd = kc & 1
                    newk = half + odd
                    cur_v = cur[:, :nt * kc * D_OUT].rearrange(
                        "p (t k f) -> p t k f", k=kc, f=D_OUT
                    )
                    if newk == 1:      # kc == 2: final add straight to f32
                        dst_v = tbuf[:, t0 * D_OUT:t1 * D_OUT].rearrange(
                            "p (t f) -> p t f", f=D_OUT
                        ).unsqueeze(2)
                    else:
                        pool = apool if toggle == 0 else bpool
                        cols = apool is pool and scra_cols or scrb_cols
                        dst = pool.tile(
                            [P, cols], F16, tag="sa" if toggle == 0 else "sb"
                        )
                        dst_v = dst[:, :nt * newk * D_OUT].rearrange(
                            "p (t k f) -> p t k f", k=newk, f=D_OUT
                        )
                    nc.vector.tensor_tensor(
                        out=dst_v[:, :, 0:half, :],
                        in0=cur_v[:, :, 0:half, :],
                        in1=cur_v[:, :, half:2 * half, :],
                        op=mybir.AluOpType.add,
                    )
                    if odd:
                        nc.vector.tensor_scalar(
                            out=dst_v[:, :, half:newk, :],
                            in0=cur_v[:, :, 2 * half:kc, :],
                            scalar1=0.0, scalar2=None,
                            op0=mybir.AluOpType.bypass,
                        )
                    if newk > 1:
                        cur, toggle = dst, 1 - toggle
                    kc = newk

                # epilogue: out = relu(s_own * sum + b)
                t3 = tbuf[:, t0 * D_OUT:t1 * D_OUT].rearrange(
                    "p (t f) -> p t f", f=D_OUT
                )
                nc.vector.tensor_tensor(
                    out=t3, in0=t3,
                    in1=sown_sb[:, t0:t1].to_broadcast([P, nt, D_OUT]),
                    op=mybir.AluOpType.mult,
                )
                if with_bias:
                    bias_b = bass.AP(
                        bias_sb[:].tensor, bias_sb[:].offset,
                        [[D_OUT, P], [0, nt], [1, D_OUT]],
                    )
                    nc.vector.tensor_tensor(
                        out=t3, in0=t3, in1=bias_b, op=mybir.AluOpType.add
                    )
                nc.scalar.activation(
                    out=obuf[:, t0 * D_OUT:t1 * D_OUT],
                    in_=tbuf[:, t0 * D_OUT:t1 * D_OUT],
                    func=mybir.ActivationFunctionType.Relu,
                )
                nc.sync.dma_start(
                    out=out[:, t0 * D_OUT:t1 * D_OUT],
                    in_=obuf[:, t0 * D_OUT:t1 * D_OUT],
                )

    nc.finalize()
    return nc


# ---------------------------------------------------------------- runner ---
def _run(inputs, trace=False):
    x, edge_index = inputs["x"], inputs["edge_index"]
    W, b = inputs["W"], inputs["b"]
    deg, node_of_pos, src_slot, dego, ktile, offs, totk = graph_prep(edge_index)

    # ---- pass 1: h' = s * (x @ W) per core
    x16 = np.concatenate(
        [np.asarray(x).astype(np.float16), np.zeros((1, D_IN), np.float16)]
    )
    w16 = np.asarray(W).astype(np.float16)
    nop_safe = np.where(node_of_pos >= 0, node_of_pos, N)
    nc1 = build_pass1()
    in1 = [
        {
            "xT": np.ascontiguousarray(
                x16[nop_safe[c * NPOS:(c + 1) * NPOS]].T
            ),
            "w": w16,
            "dego": dego[c],
        }
        for c in range(NCORES)
    ]
    res1 = bass_utils.run_bass_kernel_spmd(
        nc1, in1, core_ids=list(range(NCORES)), trace=trace
    )

    # ---- host: gather h' rows into the ELL slot layout (no float math)
    h_aug = np.zeros((N + 1, D_OUT), dtype=np.float16)
    for c in range(NCORES):
        hp = res1.results[c]["hp"].reshape(P, TPC, D_OUT)
        block = hp.transpose(1, 0, 2).reshape(NPOS, D_OUT)
        nid = node_of_pos[c * NPOS:(c + 1) * NPOS]
        m = nid >= 0
        h_aug[nid[m]] = block[m]
    slot_safe = np.where(src_slot >= 0, src_slot, N)

    # ---- pass 2: segment-sum + epilogue
    bias = np.broadcast_to(np.asarray(b).astype(np.float32), (P, D_OUT)).copy()
    with_bias = bool(np.any(np.asarray(b) != 0))
    nc2 = build_pass2(ktile, offs, totk, with_bias)
    in2 = [
        {
            "he": h_aug[slot_safe[c]].reshape(P, totk * D_OUT),
            "sown": res1.results[c]["sown"],
            "bias": bias,
        }
        for c in range(NCORES)
    ]
    res2 = bass_utils.run_bass_kernel_spmd(
        nc2, in2, core_ids=list(range(NCORES)), trace=trace
    )

    # ---- unshard (f16 -> f32 upcast is exact, not arithmetic)
    full = np.empty((N, D_OUT), dtype=np.float32)
    for c in range(NCORES):
        oc = res2.results[c]["out"].reshape(P, TPC, D_OUT).astype(np.float32)
        block = oc.transpose(1, 0, 2).reshape(NPOS, D_OUT)
        nid = node_of_pos[c * NPOS:(c + 1) * NPOS]
        m = nid >= 0
        full[nid[m]] = block[m]
    return full, [res1, res2]


def kernel(**inputs) -> np.ndarray:
    full, _ = _run(inputs, trace=False)
    return full


# revision 22
# speedup vs baseline: 1.1486x; 1.0766x over previous
"""GCN layer relu(GCNConv(x, edge_index)) on 8 Trainium2 NeuronCores.

Math (PyG GCNConv with self-loops, symmetric norm):
    deg[v]  = 1 + in-degree(v)
    s       = deg ** -0.5
    out[d]  = relu(s[d] * (sum_{e: dst(e)=d} s[src_e] * (x[src_e] @ W)) + b)

Two-launch transform-first pipeline (D_OUT=32 << D_IN=128, so messages
shrink 4x when the linear transform runs before the gather):

  Pass 1 (device): per core, its 12544 nodes' features arrive
    feature-major ([128, NPOS] fp16); 98 matmuls against W and a fused
    PSUM-drain multiply by s_own (deg**-0.5, computed on device) yield
    h' = s * (x @ W) in fp16, written back to HBM along with s_own.

  Host (index bookkeeping only): scatter h' rows to node order, gather
    them into each core's degree-sorted padded ELL slot table
    (slot 0 of each node = its self-loop).  No host float arithmetic -
    rows are moved verbatim; s[src] is already folded into h'.

  Pass 2 (device): stream the gathered messages ([128, totk*32] fp16,
    ~15 MB/core vs 56 MB/core when gathering x), segment-sum each
    node's K slots with a pairwise tensor_tensor ADD tree (2x DVE mode
    on contiguous fp16; tensor_reduce only has a 1x uop), then
    relu(s_own * sum + b) and stream out per subgroup.

Indirect DMA is deliberately avoided: TRN2's dynamic DMA honors only
one runtime offset per partition per instruction, far too slow for
1.7M edge gathers.  Replicating h' per edge costs a 4x smaller (and
perfectly sequential) HBM stream than replicating x.
"""

import math
import numpy as np

import concourse.bass as bass
import concourse.bacc as bacc
import concourse.mybir as mybir
import concourse.tile as tile
from concourse import bass_utils

# ---------------------------------------------------------------- config ---
P = 128            # partitions
D_IN = 128
D_OUT = 32
N = 100000         # nodes
E = 1600000        # edges
NCORES = 8

NPC = N // NCORES              # 12500 nodes per core
TPC = math.ceil(NPC / P)       # 98 node tiles per core
NPOS = TPC * P                 # 12544 padded positions per core
NPAD0 = NPOS - NPC             # 44 pad positions (front, degree 0)
NV = NCORES * NPOS             # padded global positions

XCH = 14                       # node tiles per pass-1 matmul/drain chunk (98 = 7*14)
DMA_COLS = 1024                # pass-2 he stream chunk (2 KiB/partition)
NTK_MAX = 256                  # max nt*K per pass-2 subgroup (16 KiB stage)

F16 = mybir.dt.float16
F32 = mybir.dt.float32


# ------------------------------------------------------------- host prep ---
def graph_prep(edge_index):
    """Degree-sorted node placement + padded ELL slot table (ints only)."""
    src = np.asarray(edge_index[0]).astype(np.int64)
    dst = np.asarray(edge_index[1]).astype(np.int64)
    deg = np.bincount(dst, minlength=N).astype(np.int64) + 1   # + self loop

    node_of_pos = np.full(NV, -1, dtype=np.int64)
    pos_of_node = np.empty(N, dtype=np.int64)
    for c in range(NCORES):
        lo = c * NPC
        order = np.argsort(deg[lo:lo + NPC], kind="stable")
        qs = c * NPOS + NPAD0 + np.arange(NPC)
        node_of_pos[qs] = lo + order
        pos_of_node[lo + order] = qs

    sdeg = np.zeros(NV, dtype=np.int64)
    valid = node_of_pos >= 0
    sdeg[valid] = deg[node_of_pos[valid]]

    # Per-tile slot count K_t (shared across cores; SPMD: one program),
    # rounded up to a multiple of 2 (cheap zero-padding, fewer K runs).
    # Runs shorter than 3 tiles are padded up into the next run: fewer,
    # larger DMAs and subgroups beat the few extra zero slots.
    ktile = sdeg.reshape(NCORES, TPC, P).max(axis=(0, 2))
    ktile = (np.maximum(ktile, 1) + 1) // 2 * 2
    while True:
        runs = []
        t0 = 0
        while t0 < TPC:
            t1 = t0 + 1
            while t1 < TPC and ktile[t1] == ktile[t0]:
                t1 += 1
            runs.append((t0, t1))
            t0 = t1
        short = [i for i, (a, b) in enumerate(runs[:-1]) if b - a < 3]
        if not short:
            break
        a, b = runs[short[0]]
        ktile[a:b] = ktile[b]          # pad up into the next (larger) run
    offs = np.concatenate([[0], np.cumsum(ktile)]).astype(np.int64)
    totk = int(offs[-1])

    # slot source table: src_slot[core][p, c] = source node of that slot
    # (-1 for padding).  Slot offs[t]+0 of node (t,p) is its self loop.
    src_slot = np.full((NCORES, P, totk), -1, dtype=np.int64)
    vreal = np.nonzero(valid)[0]
    rp = vreal % P
    rt = (vreal % NPOS) // P
    rc = vreal // NPOS
    src_slot[rc, rp, offs[rt]] = node_of_pos[vreal]          # self slots

    key = pos_of_node[dst]
    es = np.argsort(key, kind="stable")
    key_s = key[es]
    src_s = src[es]
    newrun = np.ones(E, dtype=bool)
    newrun[1:] = key_s[1:] != key_s[:-1]
    run_start = np.maximum.accumulate(np.where(newrun, np.arange(E), 0))
    kwith = np.arange(E) - run_start + 1
    ep = key_s % P
    et = (key_s % NPOS) // P
    ec = key_s // NPOS
    src_slot[ec, ep, offs[et] + kwith] = src_s

    # own-node degree per (p, t) for s = deg**-0.5 (pads get 1 -> s=1)
    dego = np.ones((NCORES, P, TPC), dtype=np.float16)
    sd = sdeg.reshape(NCORES, TPC, P)
    for c in range(NCORES):
        dego[c] = np.maximum(sd[c].T, 1).astype(np.float16)

    return deg, node_of_pos, src_slot, dego, ktile, offs, totk


def subgroups(ktile):
    """Runs of equal K, split so nt*K <= NTK_MAX (stage tile bound)."""
    sg = []
    t0 = 0
    while t0 < TPC:
        t1 = t0 + 1
        while t1 < TPC and ktile[t1] == ktile[t0]:
            t1 += 1
        k = int(ktile[t0])
        ntmax = max(1, NTK_MAX // k)
        s = t0
        while s < t1:
            e = min(t1, s + ntmax)
            sg.append((s, e, k))
            s = e
        t0 = t1
    return sg


# ------------------------------------------------------------ pass 1 ---
def build_pass1():
    """h' = s_own * (x @ W) per core; also exports s_own."""
    nc = bacc.Bacc(None, num_devices=NCORES)

    xT = nc.dram_tensor("xT", [P, NPOS], F16, kind="ExternalInput")
    w = nc.dram_tensor("w", [P, D_OUT], F16, kind="ExternalInput")
    dego = nc.dram_tensor("dego", [P, TPC], F16, kind="ExternalInput")
    hp = nc.dram_tensor("hp", [P, TPC * D_OUT], F16, kind="ExternalOutput")
    sown = nc.dram_tensor("sown", [P, TPC], F32, kind="ExternalOutput")

    pieces = [0, 28, 56, 84, TPC]   # xT arrives in 4 large DMAs

    with tile.TileContext(nc) as tc:
        with (
            tc.tile_pool(name="const", bufs=1) as cpool,
            tc.tile_pool(name="psum", bufs=7, space="PSUM") as psum_pool,
        ):
            w_sb = cpool.tile([P, D_OUT], F16)
            dego_sb = cpool.tile([P, TPC], F16)
            rtmp = cpool.tile([P, TPC], F32)
            s_own = cpool.tile([P, TPC], F32)
            hsb = cpool.tile([P, TPC * D_OUT], F16)
            xsb = cpool.tile([P, NPOS], F16)

            # Small/control DMAs ride the ACT HWDGE ring so the SP ring
            # is a pure FIFO for the big xT stream (rings are per-engine
            # FIFOs; a stalled small DMA would block everything behind it).
            nc.scalar.dma_start(out=w_sb[:], in_=w[:, :])
            nc.scalar.dma_start(out=dego_sb[:], in_=dego[:, :])
            for a, b in zip(pieces, pieces[1:]):
                nc.sync.dma_start(
                    out=xsb[:, a * P:b * P], in_=xT[:, a * P:b * P]
                )

            # s = deg ** -0.5 (recip on DVE, sqrt on ACT)
            nc.vector.reciprocal(out=rtmp[:], in_=dego_sb[:])
            nc.scalar.sqrt(out=s_own[:], in_=rtmp[:])
            nc.scalar.dma_start(out=sown[:, :], in_=s_own[:])

            for c0 in range(0, TPC, XCH):
                ch = min(XCH, TPC - c0)
                ps = psum_pool.tile([P, XCH * D_OUT], F32, tag="ps")
                for j in range(ch):
                    nc.tensor.matmul(
                        out=ps[:, j * D_OUT:(j + 1) * D_OUT],
                        lhsT=xsb[:, (c0 + j) * P:(c0 + j + 1) * P],
                        rhs=w_sb[:],
                        start=True,
                        stop=True,
                    )
                # fused PSUM drain: h' = ps * s_own  (f16 out)
                nc.vector.tensor_tensor(
                    out=hsb[:, c0 * D_OUT:(c0 + ch) * D_OUT]
                    .rearrange("p (t f) -> p t f", f=D_OUT),
                    in0=ps[:, :ch * D_OUT].rearrange("p (t f) -> p t f", f=D_OUT),
                    in1=s_own[:, c0:c0 + ch].to_broadcast([P, ch, D_OUT]),
                    op=mybir.AluOpType.mult,
                )
                if c0 + ch in pieces:
                    a = pieces[pieces.index(c0 + ch) - 1]
                    nc.scalar.dma_start(
                        out=hp[:, a * D_OUT:(c0 + ch) * D_OUT],
                        in_=hsb[:, a * D_OUT:(c0 + ch) * D_OUT],
                    )

    nc.finalize()
    return nc


# ------------------------------------------------------------ pass 2 ---
def build_pass2(ktile, offs, totk, with_bias):
    """Segment-sum gathered messages, then relu(s_own * sum + b).

    with_bias=False elides the bias adds (adding an all-zero b is an
    identity); the generic path stays available for nonzero b.
    """
    nc = bacc.Bacc(None, num_devices=NCORES)

    he = nc.dram_tensor("he", [P, totk * D_OUT], F16, kind="ExternalInput")
    sown = nc.dram_tensor("sown", [P, TPC], F32, kind="ExternalInput")
    bias = nc.dram_tensor("bias", [P, D_OUT], F32, kind="ExternalInput")
    out = nc.dram_tensor("out", [P, TPC * D_OUT], F16, kind="ExternalOutput")

    sg = subgroups(ktile)
    stg_cols = max((t1 - t0) * k for t0, t1, k in sg) * D_OUT
    scra_cols = max((t1 - t0) * (k // 2 + 1) for t0, t1, k in sg) * D_OUT
    scrb_cols = max((t1 - t0) * (k // 4 + 2) for t0, t1, k in sg) * D_OUT

    with tile.TileContext(nc) as tc:
        with (
            tc.tile_pool(name="const", bufs=1) as cpool,
            tc.tile_pool(name="stage", bufs=6) as spool,
            tc.tile_pool(name="scra", bufs=2) as apool,
            tc.tile_pool(name="scrb", bufs=2) as bpool,
        ):
            sown_sb = cpool.tile([P, TPC], F32)
            bias_sb = cpool.tile([P, D_OUT], F32)
            tbuf = cpool.tile([P, TPC * D_OUT], F32)
            obuf = cpool.tile([P, TPC * D_OUT], F16)

            # SP HWDGE ring = pure he stream; everything else (small
            # inputs, result write-backs gated on ACT relus) goes on the
            # ACT ring so a waiting write never stalls the stream FIFO.
            nc.scalar.dma_start(out=sown_sb[:], in_=sown[:, :])
            nc.scalar.dma_start(out=bias_sb[:], in_=bias[:, :])

            for (t0, t1, k) in sg:
                nt = t1 - t0
                base = int(offs[t0]) * D_OUT
                wcols = nt * k * D_OUT
                stg = spool.tile([P, stg_cols], F16, tag="stg")
                nc.sync.dma_start(
                    out=stg[:, :wcols], in_=he[:, base:base + wcols]
                )

                # pairwise halving tree over the k slots of each node
                cur, kc, toggle = stg, k, 0
                while kc > 1:
                    half = kc // 2
                    odd = kc & 1
                    newk = half + odd
                    cur_v = cur[:, :nt * kc * D_OUT].rearrange(
                        "p (t k f) -> p t k f", k=kc, f=D_OUT
                    )
                    if newk == 1:      # kc == 2: final add straight to f32
                        dst_v = tbuf[:, t0 * D_OUT:t1 * D_OUT].rearrange(
                            "p (t f) -> p t f", f=D_OUT
                        ).unsqueeze(2)
                    else:
                        pool = apool if toggle == 0 else bpool
                        cols = apool is pool and scra_cols or scrb_cols
                        dst = pool.tile(
                            [P, cols], F16, tag="sa" if toggle == 0 else "sb"
                        )
                        dst_v = dst[:, :nt * newk * D_OUT].rearrange(
                            "p (t k f) -> p t k f", k=newk, f=D_OUT
                        )
                    nc.vector.tensor_tensor(
                        out=dst_v[:, :, 0:half, :],
                        in0=cur_v[:, :, 0:half, :],
                        in1=cur_v[:, :, half:2 * half, :],
                        op=mybir.AluOpType.add,
                    )
                    if odd:
                        nc.vector.tensor_scalar(
                            out=dst_v[:, :, half:newk, :],
                            in0=cur_v[:, :, 2 * half:kc, :],
                            scalar1=0.0, scalar2=None,
                            op0=mybir.AluOpType.bypass,
                        )
                    if newk > 1:
                        cur, toggle = dst, 1 - toggle
                    kc = newk

                # epilogue: out = relu(s_own * sum + b).  Without bias the
                # whole thing is one ACT op per tile: Relu(scale*in) with
                # the per-partition scale AP (nodes live on partitions).
                if with_bias:
                    t3 = tbuf[:, t0 * D_OUT:t1 * D_OUT].rearrange(
                        "p (t f) -> p t f", f=D_OUT
                    )
                    nc.vector.tensor_tensor(
                        out=t3, in0=t3,
                        in1=sown_sb[:, t0:t1].to_broadcast([P, nt, D_OUT]),
                        op=mybir.AluOpType.mult,
                    )
                    bias_b = bass.AP(
                        bias_sb[:].tensor, bias_sb[:].offset,
                        [[D_OUT, P], [0, nt], [1, D_OUT]],
                    )
                    nc.vector.tensor_tensor(
                        out=t3, in0=t3, in1=bias_b, op=mybir.AluOpType.add
                    )
                    nc.scalar.activation(
                        out=obuf[:, t0 * D_OUT:t1 * D_OUT],
                        in_=tbuf[:, t0 * D_OUT:t1 * D_OUT],
                        func=mybir.ActivationFunctionType.Relu,
                    )
                else:
                    for t in range(t0, t1):
                        nc.scalar.activation(
                            out=obuf[:, t * D_OUT:(t + 1) * D_OUT],
                            in_=tbuf[:, t * D_OUT:(t + 1) * D_OUT],
                            func=mybir.ActivationFunctionType.Relu,
                            scale=sown_sb[:, t:t + 1],
                        )
                nc.scalar.dma_start(
                    out=out[:, t0 * D_OUT:t1 * D_OUT],
                    in_=obuf[:, t0 * D_OUT:t1 * D_OUT],
                )

    nc.finalize()
    return nc


# ---------------------------------------------------------------- runner ---
def _run(inputs, trace=False):
    x, edge_index = inputs["x"], inputs["edge_index"]
    W, b = inputs["W"], inputs["b"]
    deg, node_of_pos, src_slot, dego, ktile, offs, totk = graph_prep(edge_index)

    # ---- pass 1: h' = s * (x @ W) per core
    x16 = np.concatenate(
        [np.asarray(x).astype(np.float16), np.zeros((1, D_IN), np.float16)]
    )
    w16 = np.asarray(W).astype(np.float16)
    nop_safe = np.where(node_of_pos >= 0, node_of_pos, N)
    nc1 = build_pass1()
    in1 = [
        {
            "xT": np.ascontiguousarray(
                x16[nop_safe[c * NPOS:(c + 1) * NPOS]].T
            ),
            "w": w16,
            "dego": dego[c],
        }
        for c in range(NCORES)
    ]
    res1 = bass_utils.run_bass_kernel_spmd(
        nc1, in1, core_ids=list(range(NCORES)), trace=trace
    )

    # ---- host: gather h' rows into the ELL slot layout (no float math)
    h_aug = np.zeros((N + 1, D_OUT), dtype=np.float16)
    for c in range(NCORES):
        hp = res1.results[c]["hp"].reshape(P, TPC, D_OUT)
        block = hp.transpose(1, 0, 2).reshape(NPOS, D_OUT)
        nid = node_of_pos[c * NPOS:(c + 1) * NPOS]
        m = nid >= 0
        h_aug[nid[m]] = block[m]
    slot_safe = np.where(src_slot >= 0, src_slot, N)

    # ---- pass 2: segment-sum + epilogue
    bias = np.broadcast_to(np.asarray(b).astype(np.float32), (P, D_OUT)).copy()
    with_bias = bool(np.any(np.asarray(b) != 0))
    nc2 = build_pass2(ktile, offs, totk, with_bias)
    in2 = [
        {
            "he": h_aug[slot_safe[c]].reshape(P, totk * D_OUT),
            "sown": res1.results[c]["sown"],
            "bias": bias,
        }
        for c in range(NCORES)
    ]
    res2 = bass_utils.run_bass_kernel_spmd(
        nc2, in2, core_ids=list(range(NCORES)), trace=trace
    )

    # ---- unshard (f16 -> f32 upcast is exact, not arithmetic)
    full = np.empty((N, D_OUT), dtype=np.float32)
    for c in range(NCORES):
        oc = res2.results[c]["out"].reshape(P, TPC, D_OUT).astype(np.float32)
        block = oc.transpose(1, 0, 2).reshape(NPOS, D_OUT)
        nid = node_of_pos[c * NPOS:(c + 1) * NPOS]
        m = nid >= 0
        full[nid[m]] = block[m]
    return full, [res1, res2]


def kernel(**inputs) -> np.ndarray:
    full, _ = _run(inputs, trace=False)
    return full
